# revision 5
# baseline (speedup 1.0000x reference)
"""Viterbi CRF decode on Trainium2 (Bass), 8-core data-parallel.

Problem: B=128, S=512, T=32 (30 labels + START=30, END=31): forward
max-plus scan over S steps, backpointers, masked lengths, backward
pointer-following pass. Output [B, S] int32 tag path. Pure data parallel:
16 examples per core.

This backend's cost is dominated by a fixed per-instruction overhead
(~10-50us/instr; element count nearly free below ~2K/partition), so the
design minimizes DVE instruction count:

  - forward: 4 ops/step -- in-place add (vals = scores + part, exact
    reference association order), segmented max-reduce, drain (the
    reduce->transpose RAW hazard needs one slot of separation), and a
    single broadcast-transpose that replicates the new part vector to
    all partitions of each quadrant;
  - backpointers are NOT extracted per step: after each 64-step chunk the
    whole chunk's argmaxes are recovered in 3 batched ops (is_equal
    against the stored maxima, multiply by a descending iota, segmented
    max-reduce => first-argmax encoded as 31-i), in place over the vals
    buffer;
  - backward pointer chase is sqrt-decomposed over 32 blocks of 16 steps:
    phase A composes each block's 16 pointer LUTs into one block LUT
    (one-hot select via eq/mul/max-reduce, batched over all 32 blocks x
    16 examples per instruction); phase B resolves the 32 block-boundary
    pointers two-level (compose 4-block supers, chase the 8 super
    boundaries, then chase within supers in parallel); phase C chases all
    32 blocks' interiors in parallel (batched rounds). ~200 instrs vs
    ~1020 for the naive 2-op/step chase. All LUT math is exact
    integer-in-f32.

Layout (per core): partitions p = 32q + j; quadrant q in [0,4) holds
examples b = 4q + br (br in [0,4)); j in [0,32) is the tag index.
  FT[p, 4t+br]  = feats[b, t, j]
  TT[p, i]      = trans[i, j]
  MKF[p, 4t+br] = mask[b, t]
  SCH[p, 128u+32br+i] = feats[b, 64c+u, j] + trans[i, j]  (chunk c)
  PH[p, 4t+br]  = part_t[b, j]
Exact same fp association order as the jax reference ((feats + trans) +
part, then max), so the output is bit-exact.
"""

import numpy as np
from contextlib import ExitStack

import concourse.bass as bass
import concourse.mybir as mybir
from concourse.bass_utils import run_bass_kernel_spmd

F32 = mybir.dt.float32
AX = mybir.AxisListType
OP = mybir.AluOpType

T = 32
START = 30
END = 31
NCORES = 8


def build_nc(S, reps=1):
    assert S == 512
    K = S - 1  # 511
    nc = bass.Bass(detect_race_conditions=False)
    ft_d = nc.declare_dram_parameter("ft", [128, 4 * S], F32, isOutput=False)
    mkf_d = nc.declare_dram_parameter("mkf", [128, 4 * S + 4], F32, isOutput=False)
    tt_d = nc.declare_dram_parameter("tt", [128, 32], F32, isOutput=False)
    cst_d = nc.declare_dram_parameter("cst", [128, 64], F32, isOutput=False)
    dec_d = nc.declare_dram_parameter("dec", [128, 2048], F32, isOutput=True)

    with ExitStack() as ctx:
        def sb(name, shape, dt=F32):
            return ctx.enter_context(nc.sbuf_tensor(name, shape, dt))

        FT = sb("FT", [128, 4 * S])
        MKF = sb("MKF", [128, 4 * S + 4])
        TT = sb("TT", [128, 32])
        CST = sb("CST", [128, 64])
        SCH = sb("SCH", [128, 64 * 128])
        PH = sb("PH", [128, 4 * S + 32])
        BPW = sb("BPW", [128, 4 * S + 32])
        XS = sb("XS", [128, 4 * S + 32])
        XS2 = sb("XS2", [128, 4 * S + 32])
        ALF = sb("ALF", [128, 4 * S])
        ALB = sb("ALB", [128, 4 * S])
        P4 = sb("P4", [128, 128])
        LB0 = sb("LB0", [128, 4096])
        LB1 = sb("LB1", [128, 4096])
        E = sb("E", [128, 4096])
        SP = sb("SP", [128, 128])
        CT = sb("CT", [128, 4096])
        DECS = sb("DECS", [128, 132])
        U = sb("U", [128, 32])
        DSS = sb("DSS", [128, 36])
        DECC = sb("DECC", [128, 2048])
        TEND = sb("TEND", [128, 32])
        LPP = sb("LPP", [128, 32])
        TLP = sb("TLP", [128, 32])
        CAND = sb("CAND", [128, 32])
        MX = sb("MX", [128, 1])
        EQC = sb("EQC", [128, 32])
        SC = sb("SC", [128, 32])
        PW = sb("PW", [128, 1])
        P32 = sb("P32", [128, 32])
        T32 = sb("T32", [128, 32])
        PR = sb("PR", [128, 32])

        p4_blk = P4[:].rearrange("p (b i) -> p b i", b=4)
        LB = [LB0, LB1]

        with (
            nc.semaphore() as dma_sem,
            nc.semaphore() as done_sem,
            nc.Block() as block,
        ):
            @block.sync
            def _(sync):
                sync.dma_start(out=FT[:], in_=ft_d[:]).then_inc(dma_sem, 16)
                sync.dma_start(out=MKF[:], in_=mkf_d[:]).then_inc(dma_sem, 16)
                sync.dma_start(out=TT[:], in_=tt_d[:]).then_inc(dma_sem, 16)
                sync.dma_start(out=CST[:], in_=cst_d[:]).then_inc(dma_sem, 16)
                sync.wait_ge(done_sem, 1)
                sync.dma_start(out=dec_d[:], in_=DECC[:]).then_inc(dma_sem, 16)

            iota4_abi = CST[:, 0:32].unsqueeze(1).unsqueeze(1).broadcast_to(
                [128, 32, 4, 32])
            iota3_bi = CST[:, 0:32].unsqueeze(1).broadcast_to([128, 4, 32])
            iotad4 = CST[:, 32:64].unsqueeze(1).unsqueeze(1).broadcast_to(
                [128, 64, 4, 32])
            tt_c = TT[:].unsqueeze(1).unsqueeze(1).broadcast_to([128, 64, 4, 32])
            sch_v = SCH[:].rearrange("p (u b i) -> p u b i", b=4, i=32)
            xsg = XS[:, 0:2048].rearrange("p (g z) -> p g z", z=64)
            E4d = E[:].rearrange("p (g b i) -> p g b i", g=32, b=4)
            E3d = E[:, 0:128].rearrange("p (b x) -> p b x", x=32)
            sp3 = SP[:].rearrange("p (g b) -> p g b", b=4)
            sp4b = SP[:].rearrange("p (g b) -> p g b", b=4).unsqueeze(3).broadcast_to(
                [128, 32, 4, 32])

            def p4_build(v, t):
                v.transpose(out=p4_blk,
                            in_=PH[:, 4 * t:4 * t + 4].unsqueeze(2).broadcast_to(
                                [128, 4, 32]))

            def lb_tr(v, r):
                # LB[r%2][p=(q,x), (g,br,i)] = L_{16g+r}[(q,br), i]
                src = xsg[:, :, 4 * r:4 * r + 4].unsqueeze(3).broadcast_to(
                    [128, 32, 4, 32])
                v.transpose(out=LB[r % 2][:].rearrange(
                    "p (g b i) -> p g b i", g=32, b=4), in_=src)

            def emit_body(v):
                # ---- init ----
                v.stream_shuffle(out=TEND[:], in_=TT[:], mask=[END] * 32)
                v.tensor_scalar_add(out=PH[:, 0:4], in0=FT[:, 0:4],
                                    scalar1=TT[:, START:START + 1])
                v.tensor_sub(out=ALF[:], in0=MKF[:, 0:4 * S], in1=MKF[:, 4:4 * S + 4])
                v.tensor_scalar(out=ALB[:], in0=ALF[:], scalar1=1.0,
                                scalar2=1e30, op0=OP.subtract, op1=OP.mult)
                v.memset(DECS[:], 0.0)
                v.memset(DSS[:], 0.0)
                v.drain()
                p4_build(v, 0)

                # ---- forward + batched bp extraction ----
                for c in range(8):
                    ft_c = FT[:, 256 * c:256 * (c + 1)].rearrange(
                        "p (u b) -> p u b", b=4).unsqueeze(3).broadcast_to(
                        [128, 64, 4, 32])
                    v.tensor_tensor(out=sch_v, in0=ft_c, in1=tt_c, op=OP.add)
                    u0 = 1 if c == 0 else 0
                    for u in range(u0, 64):
                        t = 64 * c + u
                        sl = SCH[:, 128 * u:128 * u + 128]
                        v.tensor_tensor(out=sl, in0=sl, in1=P4[:], op=OP.add)
                        v.tensor_reduce(out=PH[:, 4 * t:4 * t + 4],
                                        in_=sl.rearrange("p (b i) -> p b i", b=4),
                                        axis=AX.X, op=OP.max)
                        v.drain()
                        if t < S - 1:
                            p4_build(v, t)
                    nu = 64 - u0
                    vch = SCH[:, 128 * u0:8192].rearrange(
                        "p (u b i) -> p u b i", b=4, i=32)
                    phb = PH[:, 4 * (64 * c + u0):4 * (64 * c + 64)].rearrange(
                        "p (t b) -> p t b", b=4).unsqueeze(3).broadcast_to(
                        [128, nu, 4, 32])
                    v.tensor_tensor(out=vch, in0=vch, in1=phb, op=OP.is_equal)
                    v.tensor_tensor(out=vch, in0=vch,
                                    in1=iotad4[:, 0:nu], op=OP.mult)
                    v.tensor_reduce(
                        out=BPW[:, 4 * (64 * c + u0 - 1):4 * (64 * c + 63)].rearrange(
                            "p (t b) -> p t b", b=4),
                        in_=vch, axis=AX.X, op=OP.max)

                # ---- last partition + pointer ----
                xs_bt = XS[:, 0:4 * S].rearrange("p (t b) -> p b t", b=4)
                v.tensor_tensor(out=XS[:, 0:4 * S], in0=PH[:, 0:4 * S],
                                in1=ALB[:], op=OP.add)
                v.tensor_reduce(out=LPP[:, 0:4], in_=xs_bt, axis=AX.X, op=OP.max)
                v.drain()
                v.transpose(out=TLP[:], in_=LPP[:])
                v.drain()
                v.tensor_tensor(out=CAND[:], in0=TLP[:], in1=TEND[:], op=OP.add)
                v.tensor_reduce(out=MX[:], in_=CAND[:], axis=AX.X, op=OP.max)
                v.drain()
                v.tensor_tensor(out=EQC[:], in0=CAND[:],
                                in1=MX[:].broadcast_to([128, 32]), op=OP.is_equal)
                v.tensor_tensor(out=SC[:], in0=EQC[:], in1=CST[:, 32:64],
                                op=OP.mult)
                v.tensor_reduce(out=PW[:], in_=SC[:], axis=AX.X, op=OP.max)
                v.drain()
                v.tensor_scalar(out=P32[:, 0:1], in0=PW[:], scalar1=-1.0,
                                scalar2=31.0, op0=OP.mult, op1=OP.add)
                v.drain()
                v.transpose(out=T32[:], in_=P32[:])
                v.stream_shuffle(out=PR[:], in_=T32[:], mask=[0] * 32)
                v.drain()

                # ---- decode bp + mask + at-last scatter ----
                v.tensor_scalar(out=XS2[:, 0:4 * K], in0=BPW[:, 0:4 * K],
                                scalar1=-1.0, scalar2=31.0,
                                op0=OP.mult, op1=OP.add)
                v.tensor_tensor(out=BPW[:, 0:4 * K], in0=XS2[:, 0:4 * K],
                                in1=MKF[:, 4:4 * K + 4], op=OP.mult)
                pr_b = PR[:, 0:4].unsqueeze(1).broadcast_to([128, K, 4])
                bp_v = BPW[:, 0:4 * K].rearrange("p (k b) -> p k b", b=4)
                xs_v = XS[:, 0:4 * K].rearrange("p (k b) -> p k b", b=4)
                xs2_v = XS2[:, 0:4 * K].rearrange("p (k b) -> p k b", b=4)
                alf_v = ALF[:, 0:4 * K].rearrange("p (k b) -> p k b", b=4)
                v.tensor_tensor(out=xs_v, in0=pr_b, in1=bp_v, op=OP.subtract)
                v.tensor_tensor(out=xs2_v, in0=xs_v, in1=alf_v, op=OP.mult)
                v.tensor_tensor(out=xs_v, in0=bp_v, in1=xs2_v, op=OP.add)
                # L_511 := pointer (constant LUT row, gives dec[S-1] = pointer)
                v.tensor_copy(out=XS[:, 4 * K:4 * K + 4], in_=PR[:, 0:4])

                # ---- backward phase A: compose 16-step block LUTs ----
                v.tensor_copy(out=SP[:], in_=xsg[:, :, 60:64])
                lb_tr(v, 14)
                for r in range(14, -1, -1):
                    v.tensor_tensor(out=E4d, in0=sp4b, in1=iota4_abi,
                                    op=OP.is_equal)
                    v.tensor_tensor(out=E4d, in0=E4d,
                                    in1=LB[r % 2][:].rearrange(
                                        "p (g b i) -> p g b i", g=32, b=4),
                                    op=OP.mult)
                    v.tensor_reduce(out=sp3, in_=E4d, axis=AX.X, op=OP.max)
                    if r > 0:
                        lb_tr(v, r - 1)
                v.drain()

                # ---- phase B (two-level): supers of 4 blocks ----
                # B1: compose SS_s = C_{4s} o C_{4s+1} o C_{4s+2} o C_{4s+3}
                spg = SP[:].rearrange("p (s z) -> p s z", z=16)
                u3 = U[:].rearrange("p (s b) -> p s b", b=4)
                u3b = U[:].rearrange("p (s b) -> p s b", b=4).unsqueeze(
                    3).broadcast_to([128, 8, 4, 32])
                eu4 = E[:, 0:1024].rearrange("p (s b i) -> p s b i", s=8, b=4)
                iota4_sbi = CST[:, 0:32].unsqueeze(1).unsqueeze(1).broadcast_to(
                    [128, 8, 4, 32])

                def cu_tr(v, w):
                    # CU[w%2][p=(q,x), (s,br,i)] = C_{4s+w}[(q,br), i]
                    src = spg[:, :, 4 * w:4 * w + 4].unsqueeze(3).broadcast_to(
                        [128, 8, 4, 32])
                    v.transpose(out=LB[w % 2][:, 0:1024].rearrange(
                        "p (s b i) -> p s b i", s=8, b=4), in_=src)

                v.tensor_copy(out=U[:], in_=spg[:, :, 12:16])
                cu_tr(v, 2)
                for rr in range(2, -1, -1):
                    v.tensor_tensor(out=eu4, in0=u3b, in1=iota4_sbi,
                                    op=OP.is_equal)
                    v.tensor_tensor(out=eu4, in0=eu4,
                                    in1=LB[rr % 2][:, 0:1024].rearrange(
                                        "p (s b i) -> p s b i", s=8, b=4),
                                    op=OP.mult)
                    v.tensor_reduce(out=u3, in_=eu4, axis=AX.X, op=OP.max)
                    if rr > 0:
                        cu_tr(v, rr - 1)
                v.drain()
                # B2: chase the 8 super boundaries (seed DSS[:, 32:36] = 0)
                v.transpose(out=CT[:, 0:1024].rearrange("p (c x) -> p c x", x=32),
                            in_=U[:].unsqueeze(2).broadcast_to([128, 32, 32]))
                for s in range(7, -1, -1):
                    v.tensor_tensor(
                        out=E3d,
                        in0=DSS[:, 4 * s + 4:4 * s + 8].unsqueeze(2).broadcast_to(
                            [128, 4, 32]),
                        in1=iota3_bi, op=OP.is_equal)
                    v.tensor_tensor(
                        out=E3d, in0=E3d,
                        in1=CT[:, 128 * s:128 * s + 128].rearrange(
                            "p (b x) -> p b x", x=32),
                        op=OP.mult)
                    v.tensor_reduce(out=DSS[:, 4 * s:4 * s + 4], in_=E3d,
                                    axis=AX.X, op=OP.max)
                    v.drain()
                # B3: within-super chase to all 32 block boundaries (parallel
                # over s); round w state = d_{4s+w+1}, written strided into DECS
                decsg = DECS[:, 0:128].rearrange("p (s z) -> p s z", z=16)
                cu_tr(v, 3)
                for w in range(3, -1, -1):
                    if w == 3:
                        st4 = DSS[:, 4:36].rearrange(
                            "p (s b) -> p s b", b=4).unsqueeze(3).broadcast_to(
                            [128, 8, 4, 32])
                    else:
                        st4 = decsg[:, :, 4 * (w + 1):4 * (w + 1) + 4].unsqueeze(
                            3).broadcast_to([128, 8, 4, 32])
                    v.tensor_tensor(out=eu4, in0=st4, in1=iota4_sbi,
                                    op=OP.is_equal)
                    v.tensor_tensor(out=eu4, in0=eu4,
                                    in1=LB[w % 2][:, 0:1024].rearrange(
                                        "p (s b i) -> p s b i", s=8, b=4),
                                    op=OP.mult)
                    v.tensor_reduce(out=decsg[:, :, 4 * w:4 * w + 4], in_=eu4,
                                    axis=AX.X, op=OP.max)
                    if w > 0:
                        cu_tr(v, w - 1)

                # ---- phase C: chase all block interiors in parallel ----
                for r in range(15, -1, -1):
                    lb_tr(v, r)
                    if r == 15:
                        st = DECS[:, 4:132]
                    else:
                        st = DECC[:, 128 * (r + 1):128 * (r + 2)]
                    v.tensor_tensor(
                        out=E4d,
                        in0=st.rearrange("p (g b) -> p g b", b=4).unsqueeze(3)
                        .broadcast_to([128, 32, 4, 32]),
                        in1=iota4_abi, op=OP.is_equal)
                    v.tensor_tensor(out=E4d, in0=E4d,
                                    in1=LB[r % 2][:].rearrange(
                                        "p (g b i) -> p g b i", g=32, b=4),
                                    op=OP.mult)
                    v.tensor_reduce(
                        out=DECC[:, 128 * r:128 * r + 128].rearrange(
                            "p (g b) -> p g b", b=4),
                        in_=E4d, axis=AX.X, op=OP.max)

            @block.vector
            def _(v):
                v.wait_ge(dma_sem, 64)
                for _rep in range(reps):
                    emit_body(v)
                v.drain().then_inc(done_sem, 1)

    return nc


def pack_inputs(feats, transitions, mask, S):
    """Host-side layout packing (pure data movement, no arithmetic beyond
    dtype conversion of the 0/1 mask)."""
    trans = np.ascontiguousarray(np.asarray(transitions, np.float32))
    ttrep = np.ascontiguousarray(np.tile(trans.T, (4, 1)))  # [128, 32]
    iota = np.arange(32, dtype=np.float32)
    cst = np.ascontiguousarray(
        np.tile(np.concatenate([iota, 31.0 - iota])[None, :], (128, 1)))
    in_maps = []
    bc = 16
    for c in range(NCORES):
        f = np.asarray(feats[bc * c:bc * c + bc], np.float32)  # [16, S, 32]
        ft = np.ascontiguousarray(
            f.reshape(4, 4, S, T).transpose(0, 3, 2, 1).reshape(128, 4 * S))
        m = np.asarray(mask[bc * c:bc * c + bc]).astype(np.float32)  # [16, S]
        mk = np.broadcast_to(
            m.reshape(4, 1, 4, S).transpose(0, 1, 3, 2), (4, 32, S, 4))
        mk = mk.reshape(128, 4 * S)
        mkp = np.zeros((128, 4 * S + 4), np.float32)
        mkp[:, :4 * S] = mk
        in_maps.append({"ft": ft, "mkf": mkp, "tt": ttrep, "cst": cst})
    return in_maps


def unpack_outputs(results, S):
    """DECC[32q, 128r + 4g + br] = dec[b=4q+br, k=16g+r] (values replicated
    across the 32 partitions of each quadrant; row 32q is used)."""
    out = np.empty((128, S), np.int32)
    bc = 16
    r_idx, g_idx = np.meshgrid(np.arange(16), np.arange(32), indexing="ij")
    ks = (16 * g_idx + r_idx).ravel()
    for c in range(NCORES):
        d = np.asarray(results[c]["dec"])  # [128, 2048] f32
        for q in range(4):
            row = d[32 * q]
            for br in range(4):
                cols = (128 * r_idx + 4 * g_idx + br).ravel()
                dec = np.empty(S, np.int32)
                dec[ks] = row[cols].astype(np.int32)
                out[bc * c + 4 * q + br] = dec
    return out


_NC_CACHE = {}


def kernel(feats, transitions, mask):
    B, S, Tin = feats.shape
    assert (B, Tin) == (128, 32)
    if S not in _NC_CACHE:
        _NC_CACHE[S] = build_nc(S)
    nc = _NC_CACHE[S]
    in_maps = pack_inputs(feats, transitions, mask, S)
    res = run_bass_kernel_spmd(nc, in_maps, list(range(NCORES)))
    return unpack_outputs(res.results, S)
